# revision 14
# baseline (speedup 1.0000x reference)
"""Causal self-attention (B=4, T=2048, C=768, H=12) on 8 trn2 NeuronCores.

Sharding: core = (batch b in 0..3) x (head-group g in 0..1, 6 heads each).
Each core: QKV projection for its 6 heads, causal attention, partial output
projection (its heads' rows of W_proj). Host sums the two partials per batch
and adds b_proj.

Device-side layout (per core):
  xT [768, 2048]  (host pre-transposes x[b])
  qT/kT produced as [d, t] pair tiles (lhsT = W-slice, rhs = xT)
  v produced natural [t, d] with a ones column appended per head
  S^T [k, q] = kT_block.T @ qT  -> exp on ACT -> PV: y^T += v_aug.T @ expS
    row 64 of the PV accumulator = softmax denominator (ones-column trick)
  normalize via gpsimd partition_broadcast of the reciprocal denominators
  out_partial[t, :] = sum_h yT_h.T @ Wp_h   (y^T is directly the proj lhsT)

Schedule: attention for chunk c is software-pipelined (the S matmul for
block j+1 is emitted before the PV for block j so the PE never waits on
the ACT exp), and the QKV projection for chunk c+1 plus the output
projection for chunk c-1 are injected into chunk c's block stream in
pairs of PSUM allocations (pairs preserve the S double-buffer parity).
This keeps the PE continuously busy -- which also keeps it out of the
low-frequency p-states it falls back to whenever it idles.

Engine budget: PE does all matmuls; ACT does exp plus the qkT / v /
out PSUM evictions; DVE does the causal masks and softmax
normalization arithmetic; gpsimd does the denominator broadcasts,
small repartitioning DMAs and v-tile constant columns.  The v bias
never touches the device: sum_k es*(v+bv) = PV + denom*bv, so after
normalization it contributes exactly bv @ W_proj, which the host
folds into b_proj.

Matmul operands are stored bf16 (full PE rate, half the HBM traffic);
accumulation is fp32 in PSUM; the output partials are fp32.
"""

import sys

for _p in ("/opt/pypackages", "/opt/trn_rl_repo"):
    if _p not in sys.path:
        sys.path.insert(0, _p)

import numpy as np
import ml_dtypes

import concourse.bass as bass
import concourse.tile as tile
from concourse import bacc, mybir
from concourse.bass_utils import run_bass_kernel_spmd

B, T, C, H = 4, 2048, 768, 12
HS = C // H            # 64 head dim
HPC = 6                # heads per core
GC = HPC * HS          # 384 columns per core
NCORES = 8
NK = C // 128          # 6 contraction tiles over c_in
P = 128
F32 = mybir.dt.float32
MM = mybir.dt.bfloat16   # matmul operand dtype
NP_MM = ml_dtypes.bfloat16

NQCH = T // 512        # 4 q-chunks of 512
NTB = T // P           # 16 token blocks of 128
VPB = 3 * HS           # 192 cols per head-pair block of v


def _build_nc():
    nc = bacc.Bacc("TRN2")

    xT = nc.declare_dram_parameter("xT", [C, T], MM, isOutput=False)
    wq = nc.declare_dram_parameter("wq", [C, GC], MM, isOutput=False)
    wk = nc.declare_dram_parameter("wk", [C, GC], MM, isOutput=False)
    wv = nc.declare_dram_parameter("wv", [C, GC], MM, isOutput=False)
    wp = nc.declare_dram_parameter("wp", [GC, C], MM, isOutput=False)
    bqk = nc.declare_dram_parameter("bqk", [P, 6], F32, isOutput=False)
    mask = nc.declare_dram_parameter("mask", [P, 2 * P], MM, isOutput=False)
    out = nc.declare_dram_parameter("out", [T, C], F32, isOutput=True)

    xTv = xT.ap().rearrange("(k p) t -> p k t", p=P)
    wqv = wq.ap().rearrange("(k p) d -> p k d", p=P)
    wkv = wk.ap().rearrange("(k p) d -> p k d", p=P)
    wvv = wv.ap().rearrange("(k p) d -> p k d", p=P)
    wpv = wp.ap().rearrange("(h p) n -> h p n", p=P)
    outv = out.ap().rearrange("(b p) n -> b p n", p=P)

    with tile.TileContext(nc) as tc:
        from contextlib import ExitStack

        with ExitStack() as ctx:
            pers = ctx.enter_context(tc.tile_pool(name="pers", bufs=1))
            # PSUM: psMM 2 x [128,1024] (2 banks each) + psY 4 x 1 bank = 8
            psMM = ctx.enter_context(tc.tile_pool(name="psMM", bufs=2, space="PSUM"))
            psY = ctx.enter_context(tc.tile_pool(name="psY", bufs=4, space="PSUM"))
            work = ctx.enter_context(tc.tile_pool(name="work", bufs=3))
            ytp = ctx.enter_context(tc.tile_pool(name="ytp", bufs=2))

            # ---- persistent tiles ----
            # v layout per head-PAIR block of 192 cols: [v_even(64) | ones(1) |
            # zeros(63) | v_odd(64)].  lhsT_even = cols[0:66] -> y at rows 0-63,
            # sums at row 64, zeros at 65; lhsT_odd = cols[64:192] -> sums at
            # row 0, y at rows 64-127.
            qkT = [pers.tile([P, T], MM, name=f"qkT{i}") for i in range(6)]
            vsb = [pers.tile([P, 3 * VPB], MM, name=f"v{tb}") for tb in range(NTB)]
            wph = [pers.tile([P, C], MM, name=f"wp{hp}") for hp in range(3)]
            xt_all = pers.tile([P, NK, T], MM, name="xt_all")
            wqt_all = pers.tile([P, NK, GC], MM, name="wqt_all")
            wkt_all = pers.tile([P, NK, GC], MM, name="wkt_all")
            wvt_all = pers.tile([P, NK, GC], MM, name="wvt_all")
            xt = [xt_all[:, k, :] for k in range(NK)]
            wqt = [wqt_all[:, k, :] for k in range(NK)]
            wkt = [wkt_all[:, k, :] for k in range(NK)]
            wvt = [wvt_all[:, k, :] for k in range(NK)]
            mask_sb = pers.tile([P, 2 * P], MM, name="mask")
            bqk_sb = pers.tile([P, 6], F32, name="bqk")

            # ---- DMA issue order: first what chunk-0 QKV needs.  Per-queue
            # DMA bandwidth is ~1/16 of the core total, so the big loads are
            # split per k-tile (parallel queues); SP, ACT and gpsimd dispatch
            # concurrently (a single sequencer spends ~0.6us dispatching
            # each DMA and would serialize startup).
            # SP: everything ordered by deadline; ACT: only the 13 loads
            # that gate the first matmul groups (more would delay the ACT
            # evictions behind ~0.7us-per-DMA dispatch time); gpsimd: the
            # latency-tolerant leftovers.
            for k in range(NK):
                nc.sync.dma_start(xt_all[0:64, k, 0:512],
                                  xTv[0:64, k, 0:512])
                nc.sync.dma_start(xt_all[64:P, k, 0:512],
                                  xTv[64:P, k, 0:512])
            nc.scalar.dma_start(bqk_sb[:], bqk.ap())
            for k in range(NK):
                nc.scalar.dma_start(wqt_all[:, k, 0:P], wqv[:, k, 0:P])
            for k in range(NK):
                nc.scalar.dma_start(wkt_all[:, k, 0:P], wkv[:, k, 0:P])
            for k in range(NK):
                nc.sync.dma_start(wvt_all[:, k, :], wvv[:, k, :])
            for k in range(NK):
                nc.sync.dma_start(wqt_all[:, k, P:2 * P], wqv[:, k, P:2 * P])
            for k in range(NK):
                nc.sync.dma_start(wkt_all[:, k, P:2 * P], wkv[:, k, P:2 * P])
            for k in range(NK):
                nc.sync.dma_start(xt_all[:, k, 512:1024], xTv[:, k, 512:1024])
            for k in range(NK):
                nc.sync.dma_start(wqt_all[:, k, 2 * P:GC],
                                  wqv[:, k, 2 * P:GC])
            for k in range(NK):
                nc.sync.dma_start(wkt_all[:, k, 2 * P:GC],
                                  wkv[:, k, 2 * P:GC])
            nc.gpsimd.dma_start(mask_sb[:], mask.ap())
            for c in range(2, NQCH):
                for k in range(0, NK, 2):
                    nc.sync.dma_start(
                        xt_all[:, k:k + 2, 512 * c:512 * (c + 1)],
                        xTv[:, k:k + 2, 512 * c:512 * (c + 1)],
                    )
            for hp in range(3):
                nc.gpsimd.dma_start(wph[hp][:], wpv[hp])

            # ones column for the tail's PE-broadcast norm + ACT exp-table
            # preload (the first real exp would otherwise eat the 1.3us
            # table load on the critical path)
            ones65 = pers.tile([HS + 1, HS], MM, name="ones65")
            nc.vector.memset(ones65[:], 1.0)
            # zero-weight operands for PE keep-alive matmuls: accumulating
            # 0 into the PV PSUM keeps the PE continuously busy on blocks
            # with no injected unit, which keeps it at its full-speed
            # p-state (any idle resets the frequency ramp)
            zrow = pers.tile([1, HS + 2], MM, name="zrow")
            nc.vector.memset(zrow[:], 0.0)
            drow = pers.tile([1, 512], MM, name="drow")
            nc.vector.memset(drow[:], 0.0)
            dumm = pers.tile([1, 1], F32, name="dumm")
            nc.scalar.activation(out=dumm[:], in_=bqk_sb[0:1, 0:1],
                                 func=mybir.ActivationFunctionType.Exp,
                                 scale=1.0)

            def emit_qkv_group(c, i):
                # qT / kT pair tiles: i in 0..2 -> q pair i; 3..5 -> k pair i-3
                wt = wqt if i < 3 else wkt
                p = i % 3
                ps = psMM.tile([P, 1024], F32, tag="mm", name="ps_qk")
                for k in range(NK):
                    nc.tensor.matmul(
                        ps[:, 0:512],
                        wt[k][:, P * p:P * (p + 1)],
                        xt[k][:, 512 * c:512 * (c + 1)],
                        start=(k == 0),
                        stop=(k == NK - 1),
                    )
                # bias-add eviction on the ACT engine
                with nc.allow_low_precision(reason="qkT stored bf16"):
                    nc.scalar.add(
                        qkT[i][:, 512 * c:512 * (c + 1)],
                        ps[:, 0:512],
                        bqk_sb[:, i:i + 1],
                    )

            def emit_v_block(tb):
                # v natural [t, d] + bias, packed into pair blocks; the
                # eviction arithmetic runs on gpsimd so the DVE queue stays
                # free for attention's mask multiplies
                ps = psMM.tile([P, 1024], F32, tag="mm", name="ps_v")
                for k in range(NK):
                    nc.tensor.matmul(
                        ps[:, 0:GC],
                        xt[k][:, P * tb:P * (tb + 1)],
                        wvt[k][:],
                        start=(k == 0),
                        stop=(k == NK - 1),
                    )
                v3 = vsb[tb].rearrange("p (b e) -> p b e", e=VPB)
                ps4 = ps[:, 0:GC].rearrange("p (b o d) -> p b o d", o=2, d=HS)
                with nc.allow_low_precision(reason="v stored bf16"):
                    nc.vector.tensor_copy(out=v3[:, :, 0:HS],
                                          in_=ps4[:, :, 0, :])
                    nc.vector.tensor_copy(out=v3[:, :, 2 * HS:3 * HS],
                                          in_=ps4[:, :, 1, :])
                nc.gpsimd.memset(v3[:, :, HS:HS + 1], 1.0)
                nc.gpsimd.memset(v3[:, :, HS + 1:2 * HS], 0.0)

            def emit_norm_pair(hp, ypsA, ypsB):
                # Even head: sums at ypsA row 64 -> stage to SBUF (DVE,
                # partition-aligned), DMA to partition 0 (gpsimd custom ops
                # need base-0 operands on HW), reciprocal at base 0, gpsimd
                # partition_broadcast to rows 0-63.  Odd head: sums at ypsB
                # row 0 -> reciprocal at base 0, gpsimd-broadcast across all
                # 128 rows (base-0 dst), multiply lane-aligned at base 64.
                st = work.tile([HS + 1, 512], F32, tag="st", name="st")
                nc.vector.tensor_copy(out=st[HS:HS + 1, :],
                                      in_=ypsA[HS:HS + 1, :])
                stE = work.tile([1, 512], F32, tag="stE", name="stE")
                nc.sync.dma_start(stE[:], st[HS:HS + 1, :])
                rtE = work.tile([1, 512], F32, tag="rtE", name="rtE")
                nc.vector.reciprocal_approx_fast(out=rtE[:], in_=stE[:])
                rbiA = work.tile([HS, 512], F32, tag="rbiA", name="rbiA")
                nc.gpsimd.partition_broadcast(rbiA[:], rtE[:], channels=HS)

                stB = work.tile([1, 512], F32, tag="stB", name="stB")
                nc.vector.tensor_copy(out=stB[:], in_=ypsB[0:1, :])
                rtB = work.tile([1, 512], F32, tag="rtB", name="rtB")
                nc.vector.reciprocal_approx_fast(out=rtB[:], in_=stB[:])
                rbiB = work.tile([P, 512], F32, tag="rbiB", name="rbiB")
                nc.gpsimd.partition_broadcast(rbiB[0:P, :], rtB[:],
                                              channels=P)

                yt = ytp.tile([P, 512], MM, tag=f"ytp{hp}", name=f"ytp{hp}")
                nc.vector.tensor_mul(out=yt[0:HS, :], in0=ypsA[0:HS, :],
                                     in1=rbiA[:])
                nc.vector.tensor_mul(out=yt[HS:P, :], in0=ypsB[HS:P, :],
                                     in1=rbiB[HS:P, :])
                return yt

            def emit_norm_pair_tail(hp, ypsA, ypsB):
                # PE-broadcast variant for the very end of the kernel: the
                # chain is ~2.5us shorter than the gpsimd one and there is
                # no S-stream whose PSUM rotation it could disturb
                st = work.tile([HS + 1, 512], MM, tag="stT", name="stT")
                with nc.allow_low_precision(reason="denom staged bf16"):
                    nc.vector.tensor_copy(out=st[HS:HS + 1, :],
                                          in_=ypsA[HS:HS + 1, :])
                rb = psMM.tile([P, 1024], F32, tag="mm", name="rb")
                nc.tensor.matmul(rb[0:HS, 0:512], ones65[HS:HS + 1, :],
                                 st[HS:HS + 1, :], start=True, stop=True)
                rbiA = work.tile([HS, 512], F32, tag="rbiA", name="rbiA")
                nc.vector.reciprocal_approx_fast(out=rbiA[:],
                                                 in_=rb[0:HS, 0:512])

                stB = work.tile([1, 512], F32, tag="stB", name="stB")
                nc.vector.tensor_copy(out=stB[:], in_=ypsB[0:1, :])
                rtB = work.tile([1, 512], F32, tag="rtB", name="rtB")
                nc.vector.reciprocal_approx_fast(out=rtB[:], in_=stB[:])
                rtBb = work.tile([1, 512], MM, tag="rtBb", name="rtBb")
                with nc.allow_low_precision(reason="denom staged bf16"):
                    nc.vector.tensor_copy(out=rtBb[:], in_=rtB[:])
                nc.tensor.matmul(rb[HS:P, 512:1024], ones65[0:1, :],
                                 rtBb[:], start=True, stop=True,
                                 tile_position=(0, HS))
                rbiB = work.tile([P, 512], F32, tag="rbiB", name="rbiB")
                nc.vector.tensor_copy(out=rbiB[HS:P, :],
                                      in_=rb[HS:P, 512:1024])

                yt = ytp.tile([P, 512], MM, tag="ytpT", name="ytpT")
                nc.vector.tensor_mul(out=yt[0:HS, :], in0=ypsA[0:HS, :],
                                     in1=rbiA[:])
                nc.vector.tensor_mul(out=yt[HS:P, :], in0=ypsB[HS:P, :],
                                     in1=rbiB[HS:P, :])
                return yt

            def emit_proj_tb(c, ytiles, tq):
                # one token-block of the output projection (K=128 over the
                # 3 pair tiles); exactly one psMM allocation
                tb = 4 * c + tq
                pps = psMM.tile([P, 1024], F32, tag="mm", name="ps_o")
                for hp in range(3):
                    for n0, nn in ((0, 512), (512, 256)):
                        nc.tensor.matmul(
                            pps[:, n0:n0 + nn],
                            ytiles[hp][:, P * tq:P * (tq + 1)],
                            wph[hp][:, n0:n0 + nn],
                            start=(hp == 0),
                            stop=(hp == 2),
                        )
                ot = work.tile([P, C], F32, tag="ot", name="ot")
                if tq % 2 == 0:
                    nc.scalar.copy(out=ot[:], in_=pps[:, 0:C])
                else:
                    nc.vector.tensor_copy(out=ot[:], in_=pps[:, 0:C])
                nc.sync.dma_start(outv[tb], ot[:])

            def emit_attn_chunk(c, pend, ytiles_prev, ytiles, inject):
                # pend: the previous pair's un-normalized PSUM output (hp
                # offset by -3 when it belongs to the previous chunk); its
                # norm chain is emitted under this chunk's first pair.
                # inject: emission callables, placed between blocks to keep
                # the PE fed through the exp gates (and out of its idle
                # p-state fallback).  PV runs TWO blocks behind S so its es
                # gate is always stale; a single injected unit then bridges
                # the psum-slot gate of the S matmul that follows it.
                inj = list(inject)
                jlast = 4 * c + 3
                nblocks = 3 * (jlast + 1)
                stride = max(1, nblocks // len(inj)) if inj else nblocks
                bcount = 0

                def emit_pv(ent, ypsA, ypsB):
                    pj, pqs, pes, vpj = ent
                    nc.tensor.matmul(
                        ypsA[:, pqs:512], vpj[:, 0:HS + 2],
                        pes[:, pqs:512],
                        start=(pj == 0), stop=(pj == jlast),
                    )
                    nc.tensor.matmul(
                        ypsB[:, pqs:512], vpj[:, HS:VPB],
                        pes[:, 512 + pqs:1024],
                        start=(pj == 0), stop=(pj == jlast),
                    )

                for hp in range(3):
                    qTA = qkT[hp][0:HS, :]
                    qTB = qkT[hp][HS:P, :]
                    kTA = qkT[3 + hp][0:HS, :]
                    kTB = qkT[3 + hp][HS:P, :]
                    vp = [
                        vsb[j].rearrange("p (b e) -> p b e", e=VPB)[:, hp, :]
                        for j in range(jlast + 1)
                    ]

                    ypsA = psY.tile([HS + 2, 512], F32, tag="y", name="ypsA")
                    ypsB = psY.tile([P, 512], F32, tag="y", name="ypsB")
                    pending = []
                    for j in range(jlast + 1):
                        m = j - 4 * c
                        qs = P * m if m > 0 else 0
                        sps = psMM.tile([P, 1024], F32, tag="mm", name="ps_s")
                        es = work.tile([P, 1024], MM, tag="es", name="es",
                                       bufs=5)
                        # both heads' S blocks, row-tiled (A 0-63, B 64-127)
                        nc.tensor.matmul(
                            sps[:, qs:512],
                            kTA[:, P * j:P * (j + 1)],
                            qTA[:, 512 * c + qs:512 * (c + 1)],
                            start=True, stop=True,
                        )
                        nc.tensor.matmul(
                            sps[:, 512 + qs:1024],
                            kTB[:, P * j:P * (j + 1)],
                            qTB[:, 512 * c + qs:512 * (c + 1)],
                            start=True, stop=True,
                        )
                        if qs > 0:
                            # one 3D-AP exp over both heads' [qs:512] halves
                            es2 = es.rearrange("p (u n) -> p u n", n=512)
                            sp2 = sps.rearrange("p (u n) -> p u n", n=512)
                            nc.scalar.activation(
                                out=es2[:, :, qs:512], in_=sp2[:, :, qs:512],
                                func=mybir.ActivationFunctionType.Exp,
                                scale=1.0 / 8.0)
                        else:
                            nc.scalar.activation(
                                out=es[:], in_=sps[:],
                                func=mybir.ActivationFunctionType.Exp,
                                scale=1.0 / 8.0)
                        if m >= 0:
                            # one double-wide masked multiply over both
                            # heads' diagonal sub-blocks (mask_sb [128,256])
                            es2 = es.rearrange("p (u n) -> p u n", n=512)
                            mk2 = mask_sb.rearrange("p (u n) -> p u n", n=P)
                            nc.vector.tensor_mul(
                                out=es2[:, :, qs:qs + P],
                                in0=es2[:, :, qs:qs + P], in1=mk2[:])
                        if len(pending) >= 2:
                            emit_pv(pending.pop(0), ypsA, ypsB)
                        elif pending == [] and pend is not None:
                            # previous pair's norm chain under this pair's
                            # first block so it overlaps S/exp work
                            php, pA, pB = pend
                            tgt = ytiles_prev if php < 0 else ytiles
                            tgt[php % 3] = emit_norm_pair(php % 3, pA, pB)
                            pend = None
                        pending.append((j, qs, es, vp[j]))
                        bcount += 1
                        if inj and bcount % stride == 0:
                            inj.pop(0)()
                        elif j >= 2:
                            # keep-alive: harmless zero accumulation sized
                            # to bring this block's PE time up to ~its ACT
                            # time
                            nd = min(384, 512 - qs)
                            nc.tensor.matmul(
                                ypsA[:, qs:qs + nd], zrow[:],
                                drow[:, 0:nd],
                                start=False, stop=False,
                            )
                    while pending:
                        emit_pv(pending.pop(0), ypsA, ypsB)
                    pend = (hp, ypsA, ypsB)
                # leftover injections (normally none)
                while inj:
                    inj.pop(0)()
                return pend

            # ---- main schedule ----
            # serial head: just enough QKV for chunk-0 pair-0 (its other
            # pairs' projections are injected into chunk 0's own stream)
            emit_qkv_group(0, 0)
            emit_qkv_group(0, 3)
            for tq in range(4):
                emit_v_block(tq)

            def G(c, i):
                return lambda: emit_qkv_group(c, i)

            def V(tb):
                return lambda: emit_v_block(tb)

            def PP(c, yy, tq0):
                # proj token-blocks in pairs (two psMM allocations keep the
                # S rotation parity; a single 0.65us proj unit would not
                # bridge the exp gate anyway)
                def f():
                    emit_proj_tb(c, yy, tq0)
                    emit_proj_tb(c, yy, tq0 + 1)
                return f

            pend = None
            ytiles_prev = None
            for c in range(NQCH):
                ytiles = [None] * 3
                inject = []
                # this chunk's pair-1/2 qk groups (read 4c+4 blocks in)
                inject += [G(c, 1), G(c, 4), G(c, 2), G(c, 5)]
                if c + 1 < NQCH:
                    # next chunk's pair-0 groups + its v token-blocks
                    inject += [G(c + 1, 0), G(c + 1, 3)]
                    inject += [V(4 * (c + 1) + tq) for tq in range(4)]
                if c >= 1:
                    inject += [PP(c - 1, ytiles_prev, 0),
                               PP(c - 1, ytiles_prev, 2)]
                if pend is not None:
                    # mark: the pending pair belongs to the previous chunk
                    php, pA, pB = pend
                    pend = (php - 3, pA, pB)
                pend = emit_attn_chunk(c, pend, ytiles_prev, ytiles, inject)
                ytiles_prev = ytiles

            # tail: last chunk's hp2 norm (PE-broadcast variant) + its proj
            php, pA, pB = pend
            ytiles_prev[php] = emit_norm_pair_tail(php, pA, pB)
            for tq in range(4):
                emit_proj_tb(NQCH - 1, ytiles_prev, tq)

    nc.compile()
    return nc


_nc_cache = None
last_results = None


def _get_nc():
    global _nc_cache
    if _nc_cache is None:
        _nc_cache = _build_nc()
    return _nc_cache


def make_in_maps(x, W_attn, b_attn, W_proj):
    x = np.asarray(x, np.float32)
    W_attn = np.asarray(W_attn, np.float32)
    b_attn = np.asarray(b_attn, np.float32)
    W_proj = np.asarray(W_proj, np.float32)

    kk, qq = np.meshgrid(np.arange(P), np.arange(P), indexing="ij")
    mask = np.tile((qq >= kk).astype(NP_MM), (1, 2))

    in_maps = []
    for core in range(NCORES):
        b, g = divmod(core, 2)
        hs = slice(GC * g, GC * (g + 1))
        bq = b_attn[0:C][hs]
        bk = b_attn[C:2 * C][hs]
        bqk = np.stack(
            [bq[P * p:P * (p + 1)] for p in range(3)]
            + [bk[P * p:P * (p + 1)] for p in range(3)],
            axis=1,
        ).astype(np.float32)
        in_maps.append({
            "xT": np.ascontiguousarray(x[b].T).astype(NP_MM),
            "wq": np.ascontiguousarray(W_attn[:, 0:C][:, hs]).astype(NP_MM),
            "wk": np.ascontiguousarray(W_attn[:, C:2 * C][:, hs]).astype(NP_MM),
            "wv": np.ascontiguousarray(W_attn[:, 2 * C:3 * C][:, hs]).astype(NP_MM),
            "wp": np.ascontiguousarray(W_proj[hs, :]).astype(NP_MM),
            "bqk": np.ascontiguousarray(bqk),
            "mask": mask,
        })
    return in_maps


def kernel(x, W_attn, b_attn, W_proj, b_proj, _trace=False):
    global last_results
    nc = _get_nc()
    in_maps = make_in_maps(x, W_attn, b_attn, W_proj)
    res = run_bass_kernel_spmd(nc, in_maps, list(range(NCORES)), trace=_trace)
    last_results = res
    out = np.zeros((B, T, C), np.float32)
    for core in range(NCORES):
        out[core // 2] += res.results[core]["out"]
    # v-bias contribution (sum_k es*(v+bv) normalizes to y + bv) plus b_proj
    bias = np.asarray(b_proj, np.float32) + (
        np.asarray(b_attn, np.float32)[2 * C:3 * C]
        @ np.asarray(W_proj, np.float32))
    out += bias[None, None, :]
    return out


# revision 15
# speedup vs baseline: 1.2267x; 1.2267x over previous
"""Causal self-attention (B=4, T=2048, C=768, H=12) on 8 trn2 NeuronCores.

Sharding: core = (batch b in 0..3) x (head-group g in 0..1, 6 heads each).
Each core: QKV projection for its 6 heads, causal attention, partial output
projection (its heads' rows of W_proj). Host sums the two partials per batch
and adds b_proj.

Device-side layout (per core):
  xT [768, 2048]  (host pre-transposes x[b])
  qT/kT produced as [d, t] pair tiles (lhsT = W-slice, rhs = xT)
  v produced natural [t, d] with a ones column appended per head
  S^T [k, q] = kT_block.T @ qT  -> exp on ACT -> PV: y^T += v_aug.T @ expS
    row 64 of the PV accumulator = softmax denominator (ones-column trick)
  normalize via gpsimd partition_broadcast of the reciprocal denominators
  out_partial[t, :] = sum_h yT_h.T @ Wp_h   (y^T is directly the proj lhsT)

Schedule: attention for chunk c is software-pipelined (the S matmul for
block j+1 is emitted before the PV for block j so the PE never waits on
the ACT exp), and the QKV projection for chunk c+1 plus the output
projection for chunk c-1 are injected into chunk c's block stream in
pairs of PSUM allocations (pairs preserve the S double-buffer parity).
This keeps the PE continuously busy -- which also keeps it out of the
low-frequency p-states it falls back to whenever it idles.

Engine budget: PE does all matmuls; ACT does exp plus the qkT / v /
out PSUM evictions; DVE does the causal masks and softmax
normalization arithmetic; gpsimd does the denominator broadcasts,
small repartitioning DMAs and v-tile constant columns.  The v bias
never touches the device: sum_k es*(v+bv) = PV + denom*bv, so after
normalization it contributes exactly bv @ W_proj, which the host
folds into b_proj.

Matmul operands are stored bf16 (full PE rate, half the HBM traffic);
accumulation is fp32 in PSUM; the output partials are fp32.
"""

import sys

for _p in ("/opt/pypackages", "/opt/trn_rl_repo"):
    if _p not in sys.path:
        sys.path.insert(0, _p)

import numpy as np
import ml_dtypes

import concourse.bass as bass
import concourse.tile as tile
from concourse import bacc, mybir
from concourse.bass_utils import run_bass_kernel_spmd

B, T, C, H = 4, 2048, 768, 12
HS = C // H            # 64 head dim
HPC = 6                # heads per core
GC = HPC * HS          # 384 columns per core
NCORES = 8
NK = C // 128          # 6 contraction tiles over c_in
P = 128
F32 = mybir.dt.float32
MM = mybir.dt.bfloat16   # matmul operand dtype
NP_MM = ml_dtypes.bfloat16

NQCH = T // 512        # 4 q-chunks of 512
NTB = T // P           # 16 token blocks of 128
VPB = 3 * HS           # 192 cols per head-pair block of v


def _build_nc():
    nc = bacc.Bacc("TRN2")

    xT = nc.declare_dram_parameter("xT", [C, T], MM, isOutput=False)
    wq = nc.declare_dram_parameter("wq", [C, GC], MM, isOutput=False)
    wk = nc.declare_dram_parameter("wk", [C, GC], MM, isOutput=False)
    wv = nc.declare_dram_parameter("wv", [C, GC], MM, isOutput=False)
    wp = nc.declare_dram_parameter("wp", [GC, C], MM, isOutput=False)
    bqk = nc.declare_dram_parameter("bqk", [P, 6], F32, isOutput=False)
    mask = nc.declare_dram_parameter("mask", [P, 2 * P], MM, isOutput=False)
    out = nc.declare_dram_parameter("out", [T, C], F32, isOutput=True)

    xTv = xT.ap().rearrange("(k p) t -> p k t", p=P)
    wqv = wq.ap().rearrange("(k p) d -> p k d", p=P)
    wkv = wk.ap().rearrange("(k p) d -> p k d", p=P)
    wvv = wv.ap().rearrange("(k p) d -> p k d", p=P)
    wpv = wp.ap().rearrange("(h p) n -> h p n", p=P)
    outv = out.ap().rearrange("(b p) n -> b p n", p=P)

    with tile.TileContext(nc) as tc:
        from contextlib import ExitStack

        with ExitStack() as ctx:
            pers = ctx.enter_context(tc.tile_pool(name="pers", bufs=1))
            # PSUM: psMM 2 x [128,1024] (2 banks each) + psY 4 x 1 bank = 8
            psMM = ctx.enter_context(tc.tile_pool(name="psMM", bufs=2, space="PSUM"))
            psY = ctx.enter_context(tc.tile_pool(name="psY", bufs=4, space="PSUM"))
            work = ctx.enter_context(tc.tile_pool(name="work", bufs=3))
            ytp = ctx.enter_context(tc.tile_pool(name="ytp", bufs=2))

            # ---- persistent tiles ----
            # v layout per head-PAIR block of 192 cols: [v_even(64) | ones(1) |
            # zeros(63) | v_odd(64)].  lhsT_even = cols[0:66] -> y at rows 0-63,
            # sums at row 64, zeros at 65; lhsT_odd = cols[64:192] -> sums at
            # row 0, y at rows 64-127.
            qkT = [pers.tile([P, T], MM, name=f"qkT{i}") for i in range(6)]
            vsb = [pers.tile([P, 3 * VPB], MM, name=f"v{tb}") for tb in range(NTB)]
            wph = [pers.tile([P, C], MM, name=f"wp{hp}") for hp in range(3)]
            xt_all = pers.tile([P, NK, T], MM, name="xt_all")
            wqt_all = pers.tile([P, NK, GC], MM, name="wqt_all")
            wkt_all = pers.tile([P, NK, GC], MM, name="wkt_all")
            wvt_all = pers.tile([P, NK, GC], MM, name="wvt_all")
            xt = [xt_all[:, k, :] for k in range(NK)]
            wqt = [wqt_all[:, k, :] for k in range(NK)]
            wkt = [wkt_all[:, k, :] for k in range(NK)]
            wvt = [wvt_all[:, k, :] for k in range(NK)]
            mask_sb = pers.tile([P, 2 * P], MM, name="mask")
            bqk_sb = pers.tile([P, 6], F32, name="bqk")

            # ---- DMA issue order: first what chunk-0 QKV needs.  Per-queue
            # DMA bandwidth is ~1/16 of the core total, so the big loads are
            # split per k-tile (parallel queues); SP, ACT and gpsimd dispatch
            # concurrently (a single sequencer spends ~0.6us dispatching
            # each DMA and would serialize startup).
            # SP: everything ordered by deadline; ACT: only the 13 loads
            # that gate the first matmul groups (more would delay the ACT
            # evictions behind ~0.7us-per-DMA dispatch time); gpsimd: the
            # latency-tolerant leftovers.
            for k in range(NK):
                nc.sync.dma_start(xt_all[0:64, k, 0:512],
                                  xTv[0:64, k, 0:512])
                nc.sync.dma_start(xt_all[64:P, k, 0:512],
                                  xTv[64:P, k, 0:512])
            nc.scalar.dma_start(bqk_sb[:], bqk.ap())
            for k in range(NK):
                nc.scalar.dma_start(wqt_all[:, k, 0:P], wqv[:, k, 0:P])
            for k in range(NK):
                nc.scalar.dma_start(wkt_all[:, k, 0:P], wkv[:, k, 0:P])
            for k in range(NK):
                nc.sync.dma_start(wvt_all[:, k, :], wvv[:, k, :])
            for k in range(NK):
                nc.sync.dma_start(wqt_all[:, k, P:2 * P], wqv[:, k, P:2 * P])
            for k in range(NK):
                nc.sync.dma_start(wkt_all[:, k, P:2 * P], wkv[:, k, P:2 * P])
            for k in range(NK):
                nc.sync.dma_start(xt_all[:, k, 512:1024], xTv[:, k, 512:1024])
            for k in range(NK):
                nc.sync.dma_start(wqt_all[:, k, 2 * P:GC],
                                  wqv[:, k, 2 * P:GC])
            for k in range(NK):
                nc.sync.dma_start(wkt_all[:, k, 2 * P:GC],
                                  wkv[:, k, 2 * P:GC])
            nc.gpsimd.dma_start(mask_sb[:], mask.ap())
            for c in range(2, NQCH):
                for k in range(0, NK, 2):
                    nc.sync.dma_start(
                        xt_all[:, k:k + 2, 512 * c:512 * (c + 1)],
                        xTv[:, k:k + 2, 512 * c:512 * (c + 1)],
                    )
            for hp in range(3):
                nc.gpsimd.dma_start(wph[hp][:], wpv[hp])

            # ones column for the tail's PE-broadcast norm + ACT exp-table
            # preload (the first real exp would otherwise eat the 1.3us
            # table load on the critical path)
            ones65 = pers.tile([HS + 1, HS], MM, name="ones65")
            nc.vector.memset(ones65[:], 1.0)
            dumm = pers.tile([1, 1], F32, name="dumm")
            nc.scalar.activation(out=dumm[:], in_=bqk_sb[0:1, 0:1],
                                 func=mybir.ActivationFunctionType.Exp,
                                 scale=1.0)

            def emit_qkv_group_mm(c, i):
                # qT / kT pair tiles: i in 0..2 -> q pair i; 3..5 -> k pair i-3
                wt = wqt if i < 3 else wkt
                p = i % 3
                ps = psMM.tile([P, 1024], F32, tag="mm", name="ps_qk")
                for k in range(NK):
                    nc.tensor.matmul(
                        ps[:, 0:512],
                        wt[k][:, P * p:P * (p + 1)],
                        xt[k][:, 512 * c:512 * (c + 1)],
                        start=(k == 0),
                        stop=(k == NK - 1),
                    )
                return ps

            def emit_qkv_evict(c, i, ps):
                # bias-add eviction on DVE (keeps ACT free for exp); when
                # deferred a block behind the matmuls it never stalls
                with nc.allow_low_precision(reason="qkT stored bf16"):
                    nc.vector.tensor_scalar_add(
                        out=qkT[i][:, 512 * c:512 * (c + 1)],
                        in0=ps[:, 0:512],
                        scalar1=bqk_sb[:, i:i + 1],
                    )

            def emit_qkv_group(c, i):
                ps = emit_qkv_group_mm(c, i)
                emit_qkv_evict(c, i, ps)

            def emit_v_block(tb):
                # v natural [t, d] + bias, packed into pair blocks; the
                # eviction arithmetic runs on gpsimd so the DVE queue stays
                # free for attention's mask multiplies
                ps = psMM.tile([P, 1024], F32, tag="mm", name="ps_v")
                for k in range(NK):
                    nc.tensor.matmul(
                        ps[:, 0:GC],
                        xt[k][:, P * tb:P * (tb + 1)],
                        wvt[k][:],
                        start=(k == 0),
                        stop=(k == NK - 1),
                    )
                v3 = vsb[tb].rearrange("p (b e) -> p b e", e=VPB)
                ps4 = ps[:, 0:GC].rearrange("p (b o d) -> p b o d", o=2, d=HS)
                with nc.allow_low_precision(reason="v stored bf16"):
                    nc.vector.tensor_copy(out=v3[:, :, 0:HS],
                                          in_=ps4[:, :, 0, :])
                    nc.vector.tensor_copy(out=v3[:, :, 2 * HS:3 * HS],
                                          in_=ps4[:, :, 1, :])
                nc.gpsimd.memset(v3[:, :, HS:HS + 1], 1.0)
                nc.gpsimd.memset(v3[:, :, HS + 1:2 * HS], 0.0)

            def emit_norm_pair(hp, ypsA, ypsB):
                # Even head: sums at ypsA row 64 -> stage to SBUF (DVE,
                # partition-aligned), DMA to partition 0 (gpsimd custom ops
                # need base-0 operands on HW), reciprocal at base 0, gpsimd
                # partition_broadcast to rows 0-63.  Odd head: sums at ypsB
                # row 0 -> reciprocal at base 0, gpsimd-broadcast across all
                # 128 rows (base-0 dst), multiply lane-aligned at base 64.
                st = work.tile([HS + 1, 512], F32, tag="st", name="st")
                nc.vector.tensor_copy(out=st[HS:HS + 1, :],
                                      in_=ypsA[HS:HS + 1, :])
                stE = work.tile([1, 512], F32, tag="stE", name="stE")
                nc.sync.dma_start(stE[:], st[HS:HS + 1, :])
                rtE = work.tile([1, 512], F32, tag="rtE", name="rtE")
                nc.vector.reciprocal_approx_fast(out=rtE[:], in_=stE[:])
                rbiA = work.tile([HS, 512], F32, tag="rbiA", name="rbiA")
                nc.gpsimd.partition_broadcast(rbiA[:], rtE[:], channels=HS)

                stB = work.tile([1, 512], F32, tag="stB", name="stB")
                nc.vector.tensor_copy(out=stB[:], in_=ypsB[0:1, :])
                rtB = work.tile([1, 512], F32, tag="rtB", name="rtB")
                nc.vector.reciprocal_approx_fast(out=rtB[:], in_=stB[:])
                rbiB = work.tile([P, 512], F32, tag="rbiB", name="rbiB")
                nc.gpsimd.partition_broadcast(rbiB[0:P, :], rtB[:],
                                              channels=P)

                yt = ytp.tile([P, 512], MM, tag=f"ytp{hp}", name=f"ytp{hp}")
                nc.vector.tensor_mul(out=yt[0:HS, :], in0=ypsA[0:HS, :],
                                     in1=rbiA[:])
                nc.vector.tensor_mul(out=yt[HS:P, :], in0=ypsB[HS:P, :],
                                     in1=rbiB[HS:P, :])
                return yt

            def emit_norm_pair_tail(hp, ypsA, ypsB):
                # PE-broadcast variant for the very end of the kernel: the
                # chain is ~2.5us shorter than the gpsimd one and there is
                # no S-stream whose PSUM rotation it could disturb
                st = work.tile([HS + 1, 512], MM, tag="stT", name="stT")
                with nc.allow_low_precision(reason="denom staged bf16"):
                    nc.vector.tensor_copy(out=st[HS:HS + 1, :],
                                          in_=ypsA[HS:HS + 1, :])
                rb = psMM.tile([P, 1024], F32, tag="mm", name="rb")
                nc.tensor.matmul(rb[0:HS, 0:512], ones65[HS:HS + 1, :],
                                 st[HS:HS + 1, :], start=True, stop=True)
                rbiA = work.tile([HS, 512], F32, tag="rbiA", name="rbiA")
                nc.vector.reciprocal_approx_fast(out=rbiA[:],
                                                 in_=rb[0:HS, 0:512])

                stB = work.tile([1, 512], F32, tag="stB", name="stB")
                nc.vector.tensor_copy(out=stB[:], in_=ypsB[0:1, :])
                rtB = work.tile([1, 512], F32, tag="rtB", name="rtB")
                nc.vector.reciprocal_approx_fast(out=rtB[:], in_=stB[:])
                rtBb = work.tile([1, 512], MM, tag="rtBb", name="rtBb")
                with nc.allow_low_precision(reason="denom staged bf16"):
                    nc.vector.tensor_copy(out=rtBb[:], in_=rtB[:])
                nc.tensor.matmul(rb[HS:P, 512:1024], ones65[0:1, :],
                                 rtBb[:], start=True, stop=True,
                                 tile_position=(0, HS))
                rbiB = work.tile([P, 512], F32, tag="rbiB", name="rbiB")
                nc.vector.tensor_copy(out=rbiB[HS:P, :],
                                      in_=rb[HS:P, 512:1024])

                yt = ytp.tile([P, 512], MM, tag="ytpT", name="ytpT")
                nc.vector.tensor_mul(out=yt[0:HS, :], in0=ypsA[0:HS, :],
                                     in1=rbiA[:])
                nc.vector.tensor_mul(out=yt[HS:P, :], in0=ypsB[HS:P, :],
                                     in1=rbiB[HS:P, :])
                return yt

            def emit_proj_mm(c, ytiles, tq):
                # one token-block of the output projection (K=128 over the
                # 3 pair tiles); exactly one psMM allocation
                pps = psMM.tile([P, 1024], F32, tag="mm", name="ps_o")
                for hp in range(3):
                    for n0, nn in ((0, 512), (512, 256)):
                        nc.tensor.matmul(
                            pps[:, n0:n0 + nn],
                            ytiles[hp][:, P * tq:P * (tq + 1)],
                            wph[hp][:, n0:n0 + nn],
                            start=(hp == 0),
                            stop=(hp == 2),
                        )
                return pps

            def emit_proj_evict(c, tq, pps, on_act=False):
                tb = 4 * c + tq
                ot = work.tile([P, C], F32, tag="ot", name="ot")
                if on_act:
                    nc.scalar.copy(out=ot[:], in_=pps[:, 0:C])
                else:
                    nc.vector.tensor_copy(out=ot[:], in_=pps[:, 0:C])
                nc.sync.dma_start(outv[tb], ot[:])

            def emit_proj_tb(c, ytiles, tq, on_act=False):
                pps = emit_proj_mm(c, ytiles, tq)
                emit_proj_evict(c, tq, pps, on_act)

            def emit_attn_chunk(c, pend, ytiles_prev, ytiles, inject):
                # pend: the previous pair's un-normalized PSUM output (hp
                # offset by -3 when it belongs to the previous chunk); its
                # norm chain is emitted under this chunk's first pair.
                # inject: emission callables, placed between blocks to keep
                # the PE fed through the exp gates (and out of its idle
                # p-state fallback).  PV runs TWO blocks behind S so its es
                # gate is always stale; a single injected unit then bridges
                # the psum-slot gate of the S matmul that follows it.
                inj = list(inject)
                pend_ev = []
                jlast = 4 * c + 3
                nblocks = 3 * (jlast + 1)
                stride = max(1, nblocks // len(inj)) if inj else nblocks
                bcount = 0

                def emit_pv(ent, ypsA, ypsB):
                    pj, pqs, pes, vpj = ent
                    nc.tensor.matmul(
                        ypsA[:, pqs:512], vpj[:, 0:HS + 2],
                        pes[:, pqs:512],
                        start=(pj == 0), stop=(pj == jlast),
                    )
                    nc.tensor.matmul(
                        ypsB[:, pqs:512], vpj[:, HS:VPB],
                        pes[:, 512 + pqs:1024],
                        start=(pj == 0), stop=(pj == jlast),
                    )

                for hp in range(3):
                    qTA = qkT[hp][0:HS, :]
                    qTB = qkT[hp][HS:P, :]
                    kTA = qkT[3 + hp][0:HS, :]
                    kTB = qkT[3 + hp][HS:P, :]
                    vp = [
                        vsb[j].rearrange("p (b e) -> p b e", e=VPB)[:, hp, :]
                        for j in range(jlast + 1)
                    ]

                    ypsA = psY.tile([HS + 2, 512], F32, tag="y", name="ypsA")
                    ypsB = psY.tile([P, 512], F32, tag="y", name="ypsB")
                    pending = []
                    for j in range(jlast + 1):
                        m = j - 4 * c
                        qs = P * m if m > 0 else 0
                        sps = psMM.tile([P, 1024], F32, tag="mm", name="ps_s")
                        es = work.tile([P, 1024], MM, tag="es", name="es",
                                       bufs=5)
                        # both heads' S blocks, row-tiled (A 0-63, B 64-127)
                        nc.tensor.matmul(
                            sps[:, qs:512],
                            kTA[:, P * j:P * (j + 1)],
                            qTA[:, 512 * c + qs:512 * (c + 1)],
                            start=True, stop=True,
                        )
                        nc.tensor.matmul(
                            sps[:, 512 + qs:1024],
                            kTB[:, P * j:P * (j + 1)],
                            qTB[:, 512 * c + qs:512 * (c + 1)],
                            start=True, stop=True,
                        )
                        if qs > 0:
                            # one 3D-AP exp over both heads' [qs:512] halves
                            es2 = es.rearrange("p (u n) -> p u n", n=512)
                            sp2 = sps.rearrange("p (u n) -> p u n", n=512)
                            nc.scalar.activation(
                                out=es2[:, :, qs:512], in_=sp2[:, :, qs:512],
                                func=mybir.ActivationFunctionType.Exp,
                                scale=1.0 / 8.0)
                        else:
                            nc.scalar.activation(
                                out=es[:], in_=sps[:],
                                func=mybir.ActivationFunctionType.Exp,
                                scale=1.0 / 8.0)
                        if m >= 0:
                            # one double-wide masked multiply over both
                            # heads' diagonal sub-blocks (mask_sb [128,256])
                            es2 = es.rearrange("p (u n) -> p u n", n=512)
                            mk2 = mask_sb.rearrange("p (u n) -> p u n", n=P)
                            nc.vector.tensor_mul(
                                out=es2[:, :, qs:qs + P],
                                in0=es2[:, :, qs:qs + P], in1=mk2[:])
                        if len(pending) >= 2:
                            emit_pv(pending.pop(0), ypsA, ypsB)
                        elif pending == [] and pend is not None:
                            # previous pair's norm chain under this pair's
                            # first block so it overlaps S/exp work
                            php, pA, pB = pend
                            tgt = ytiles_prev if php < 0 else ytiles
                            tgt[php % 3] = emit_norm_pair(php % 3, pA, pB)
                            pend = None
                        pending.append((j, qs, es, vp[j]))
                        bcount += 1
                        # flush the previous injected unit's eviction: one
                        # block behind its matmuls it never head-of-line
                        # blocks the evicting engine
                        while pend_ev:
                            pend_ev.pop(0)()
                        if inj and bcount % stride == 0:
                            ev = inj.pop(0)()
                            if ev is not None:
                                pend_ev.append(ev)
                    while pending:
                        emit_pv(pending.pop(0), ypsA, ypsB)
                    pend = (hp, ypsA, ypsB)
                # leftover injections (normally none)
                while inj:
                    ev = inj.pop(0)()
                    if ev is not None:
                        pend_ev.append(ev)
                while pend_ev:
                    pend_ev.pop(0)()
                return pend

            # ---- main schedule ----
            # serial head: just enough QKV for chunk-0 pair-0 (its other
            # pairs' projections are injected into chunk 0's own stream)
            emit_qkv_group(0, 0)
            emit_qkv_group(0, 3)
            for tq in range(4):
                emit_v_block(tq)

            def G(c, i):
                def f():
                    ps = emit_qkv_group_mm(c, i)
                    return lambda: emit_qkv_evict(c, i, ps)
                return f

            def V(tb):
                def f():
                    emit_v_block(tb)
                    return None
                return f

            def PP(c, yy, tq0):
                # proj token-blocks in pairs (two psMM allocations keep the
                # S rotation parity)
                def f():
                    p1 = emit_proj_mm(c, yy, tq0)
                    p2 = emit_proj_mm(c, yy, tq0 + 1)

                    def ev():
                        emit_proj_evict(c, tq0, p1)
                        emit_proj_evict(c, tq0 + 1, p2)
                    return ev
                return f

            pend = None
            ytiles_prev = None
            for c in range(NQCH):
                ytiles = [None] * 3
                inject = []
                # this chunk's pair-1/2 qk groups (read 4c+4 blocks in)
                inject += [G(c, 1), G(c, 4), G(c, 2), G(c, 5)]
                if c + 1 < NQCH:
                    # next chunk's pair-0 groups + its v token-blocks
                    inject += [G(c + 1, 0), G(c + 1, 3)]
                    inject += [V(4 * (c + 1) + tq) for tq in range(4)]
                if c >= 1:
                    inject += [PP(c - 1, ytiles_prev, 0),
                               PP(c - 1, ytiles_prev, 2)]
                if pend is not None:
                    # mark: the pending pair belongs to the previous chunk
                    php, pA, pB = pend
                    pend = (php - 3, pA, pB)
                pend = emit_attn_chunk(c, pend, ytiles_prev, ytiles, inject)
                ytiles_prev = ytiles

            # tail: last chunk's hp2 norm (PE-broadcast variant) + its proj
            php, pA, pB = pend
            ytiles_prev[php] = emit_norm_pair_tail(php, pA, pB)
            for tq in range(4):
                emit_proj_tb(NQCH - 1, ytiles_prev, tq, on_act=(tq % 2 == 0))

    nc.compile()
    return nc


_nc_cache = None
last_results = None


def _get_nc():
    global _nc_cache
    if _nc_cache is None:
        _nc_cache = _build_nc()
    return _nc_cache


def make_in_maps(x, W_attn, b_attn, W_proj):
    x = np.asarray(x, np.float32)
    W_attn = np.asarray(W_attn, np.float32)
    b_attn = np.asarray(b_attn, np.float32)
    W_proj = np.asarray(W_proj, np.float32)

    kk, qq = np.meshgrid(np.arange(P), np.arange(P), indexing="ij")
    mask = np.tile((qq >= kk).astype(NP_MM), (1, 2))

    in_maps = []
    for core in range(NCORES):
        b, g = divmod(core, 2)
        hs = slice(GC * g, GC * (g + 1))
        bq = b_attn[0:C][hs]
        bk = b_attn[C:2 * C][hs]
        bqk = np.stack(
            [bq[P * p:P * (p + 1)] for p in range(3)]
            + [bk[P * p:P * (p + 1)] for p in range(3)],
            axis=1,
        ).astype(np.float32)
        in_maps.append({
            "xT": np.ascontiguousarray(x[b].T).astype(NP_MM),
            "wq": np.ascontiguousarray(W_attn[:, 0:C][:, hs]).astype(NP_MM),
            "wk": np.ascontiguousarray(W_attn[:, C:2 * C][:, hs]).astype(NP_MM),
            "wv": np.ascontiguousarray(W_attn[:, 2 * C:3 * C][:, hs]).astype(NP_MM),
            "wp": np.ascontiguousarray(W_proj[hs, :]).astype(NP_MM),
            "bqk": np.ascontiguousarray(bqk),
            "mask": mask,
        })
    return in_maps


def kernel(x, W_attn, b_attn, W_proj, b_proj, _trace=False):
    global last_results
    nc = _get_nc()
    in_maps = make_in_maps(x, W_attn, b_attn, W_proj)
    res = run_bass_kernel_spmd(nc, in_maps, list(range(NCORES)), trace=_trace)
    last_results = res
    out = np.zeros((B, T, C), np.float32)
    for core in range(NCORES):
        out[core // 2] += res.results[core]["out"]
    # v-bias contribution (sum_k es*(v+bv) normalizes to y + bv) plus b_proj
    bias = np.asarray(b_proj, np.float32) + (
        np.asarray(b_attn, np.float32)[2 * C:3 * C]
        @ np.asarray(W_proj, np.float32))
    out += bias[None, None, :]
    return out


# revision 16
# speedup vs baseline: 1.2379x; 1.0091x over previous
"""Causal self-attention (B=4, T=2048, C=768, H=12) on 8 trn2 NeuronCores.

Sharding: core = (batch b in 0..3) x (head-group g in 0..1, 6 heads each).
Each core: QKV projection for its 6 heads, causal attention, partial output
projection (its heads' rows of W_proj). Host sums the two partials per batch
and adds b_proj.

Device-side layout (per core):
  xT [768, 2048]  (host pre-transposes x[b])
  qT/kT produced as [d, t] pair tiles (lhsT = W-slice, rhs = xT)
  v produced natural [t, d] with a ones column appended per head
  S^T [k, q] = kT_block.T @ qT  -> exp on ACT -> PV: y^T += v_aug.T @ expS
    row 64 of the PV accumulator = softmax denominator (ones-column trick)
  normalize via gpsimd partition_broadcast of the reciprocal denominators
  out_partial[t, :] = sum_h yT_h.T @ Wp_h   (y^T is directly the proj lhsT)

Schedule: attention for chunk c is software-pipelined (the S matmul for
block j+1 is emitted before the PV for block j so the PE never waits on
the ACT exp), and the QKV projection for chunk c+1 plus the output
projection for chunk c-1 are injected into chunk c's block stream in
pairs of PSUM allocations (pairs preserve the S double-buffer parity).
This keeps the PE continuously busy -- which also keeps it out of the
low-frequency p-states it falls back to whenever it idles.

Engine budget: PE does all matmuls; ACT does exp plus the qkT / v /
out PSUM evictions; DVE does the causal masks and softmax
normalization arithmetic; gpsimd does the denominator broadcasts,
small repartitioning DMAs and v-tile constant columns.  The v bias
never touches the device: sum_k es*(v+bv) = PV + denom*bv, so after
normalization it contributes exactly bv @ W_proj, which the host
folds into b_proj.

Matmul operands are stored bf16 (full PE rate, half the HBM traffic);
accumulation is fp32 in PSUM; the output partials are fp32.
"""

import sys

for _p in ("/opt/pypackages", "/opt/trn_rl_repo"):
    if _p not in sys.path:
        sys.path.insert(0, _p)

import numpy as np
import ml_dtypes

import concourse.bass as bass
import concourse.tile as tile
from concourse import bacc, mybir
from concourse.bass_utils import run_bass_kernel_spmd

B, T, C, H = 4, 2048, 768, 12
HS = C // H            # 64 head dim
HPC = 6                # heads per core
GC = HPC * HS          # 384 columns per core
NCORES = 8
NK = C // 128          # 6 contraction tiles over c_in
P = 128
F32 = mybir.dt.float32
MM = mybir.dt.bfloat16   # matmul operand dtype
NP_MM = ml_dtypes.bfloat16

NQCH = T // 512        # 4 q-chunks of 512
NTB = T // P           # 16 token blocks of 128
VPB = 3 * HS           # 192 cols per head-pair block of v


def _build_nc():
    nc = bacc.Bacc("TRN2")

    xT = nc.declare_dram_parameter("xT", [C, T], MM, isOutput=False)
    wq = nc.declare_dram_parameter("wq", [C, GC], MM, isOutput=False)
    wk = nc.declare_dram_parameter("wk", [C, GC], MM, isOutput=False)
    wv = nc.declare_dram_parameter("wv", [C, GC], MM, isOutput=False)
    wp = nc.declare_dram_parameter("wp", [GC, C], MM, isOutput=False)
    bqk = nc.declare_dram_parameter("bqk", [P, 6], F32, isOutput=False)
    mask = nc.declare_dram_parameter("mask", [P, 2 * P], MM, isOutput=False)
    out = nc.declare_dram_parameter("out", [T, C], F32, isOutput=True)

    xTv = xT.ap().rearrange("(k p) t -> p k t", p=P)
    wqv = wq.ap().rearrange("(k p) d -> p k d", p=P)
    wkv = wk.ap().rearrange("(k p) d -> p k d", p=P)
    wvv = wv.ap().rearrange("(k p) d -> p k d", p=P)
    wpv = wp.ap().rearrange("(h p) n -> h p n", p=P)
    outv = out.ap().rearrange("(b p) n -> b p n", p=P)

    with tile.TileContext(nc) as tc:
        from contextlib import ExitStack

        with ExitStack() as ctx:
            pers = ctx.enter_context(tc.tile_pool(name="pers", bufs=1))
            # PSUM: psMM 2 x [128,1024] (2 banks each) + psY 4 x 1 bank = 8
            psMM = ctx.enter_context(tc.tile_pool(name="psMM", bufs=2, space="PSUM"))
            psY = ctx.enter_context(tc.tile_pool(name="psY", bufs=4, space="PSUM"))
            work = ctx.enter_context(tc.tile_pool(name="work", bufs=3))
            ytp = ctx.enter_context(tc.tile_pool(name="ytp", bufs=2))

            # ---- persistent tiles ----
            # v layout per head-PAIR block of 192 cols: [v_even(64) | ones(1) |
            # zeros(63) | v_odd(64)].  lhsT_even = cols[0:66] -> y at rows 0-63,
            # sums at row 64, zeros at 65; lhsT_odd = cols[64:192] -> sums at
            # row 0, y at rows 64-127.
            qkT = [pers.tile([P, T], MM, name=f"qkT{i}") for i in range(6)]
            vsb = [pers.tile([P, 3 * VPB], MM, name=f"v{tb}") for tb in range(NTB)]
            wph = [pers.tile([P, C], MM, name=f"wp{hp}") for hp in range(3)]
            xt_all = pers.tile([P, NK, T], MM, name="xt_all")
            wqt_all = pers.tile([P, NK, GC], MM, name="wqt_all")
            wkt_all = pers.tile([P, NK, GC], MM, name="wkt_all")
            wvt_all = pers.tile([P, NK, GC], MM, name="wvt_all")
            xt = [xt_all[:, k, :] for k in range(NK)]
            wqt = [wqt_all[:, k, :] for k in range(NK)]
            wkt = [wkt_all[:, k, :] for k in range(NK)]
            wvt = [wvt_all[:, k, :] for k in range(NK)]
            mask_sb = pers.tile([P, 2 * P], MM, name="mask")
            bqk_sb = pers.tile([P, 6], F32, name="bqk")

            # ---- DMA issue order: first what chunk-0 QKV needs.  Per-queue
            # DMA bandwidth is ~1/16 of the core total, so the big loads are
            # split per k-tile (parallel queues); SP, ACT and gpsimd dispatch
            # concurrently (a single sequencer spends ~0.6us dispatching
            # each DMA and would serialize startup).
            # SP: everything ordered by deadline; ACT: only the 13 loads
            # that gate the first matmul groups (more would delay the ACT
            # evictions behind ~0.7us-per-DMA dispatch time); gpsimd: the
            # latency-tolerant leftovers.
            for k in range(NK):
                nc.sync.dma_start(xt_all[0:64, k, 0:512],
                                  xTv[0:64, k, 0:512])
                nc.sync.dma_start(xt_all[64:P, k, 0:512],
                                  xTv[64:P, k, 0:512])
            nc.scalar.dma_start(bqk_sb[:], bqk.ap())
            for k in range(NK):
                nc.scalar.dma_start(wqt_all[:, k, 0:P], wqv[:, k, 0:P])
            for k in range(NK):
                nc.scalar.dma_start(wkt_all[:, k, 0:P], wkv[:, k, 0:P])
            for k in range(NK):
                nc.sync.dma_start(wvt_all[:, k, :], wvv[:, k, :])
            for k in range(NK):
                nc.sync.dma_start(wqt_all[:, k, P:2 * P], wqv[:, k, P:2 * P])
            for k in range(NK):
                nc.sync.dma_start(wkt_all[:, k, P:2 * P], wkv[:, k, P:2 * P])
            for k in range(NK):
                nc.sync.dma_start(xt_all[:, k, 512:1024], xTv[:, k, 512:1024])
            for k in range(NK):
                nc.sync.dma_start(wqt_all[:, k, 2 * P:GC],
                                  wqv[:, k, 2 * P:GC])
            for k in range(NK):
                nc.sync.dma_start(wkt_all[:, k, 2 * P:GC],
                                  wkv[:, k, 2 * P:GC])
            nc.gpsimd.dma_start(mask_sb[:], mask.ap())
            for c in range(2, NQCH):
                for k in range(0, NK, 2):
                    nc.sync.dma_start(
                        xt_all[:, k:k + 2, 512 * c:512 * (c + 1)],
                        xTv[:, k:k + 2, 512 * c:512 * (c + 1)],
                    )
            for hp in range(3):
                nc.gpsimd.dma_start(wph[hp][:], wpv[hp])

            # ones column for the tail's PE-broadcast norm + ACT exp-table
            # preload (the first real exp would otherwise eat the 1.3us
            # table load on the critical path)
            ones65 = pers.tile([HS + 1, HS], MM, name="ones65")
            nc.vector.memset(ones65[:], 1.0)
            dumm = pers.tile([1, 1], F32, name="dumm")
            nc.scalar.activation(out=dumm[:], in_=bqk_sb[0:1, 0:1],
                                 func=mybir.ActivationFunctionType.Exp,
                                 scale=1.0)

            def emit_qkv_group_mm(c, i):
                # qT / kT pair tiles: i in 0..2 -> q pair i; 3..5 -> k pair i-3
                wt = wqt if i < 3 else wkt
                p = i % 3
                ps = psMM.tile([P, 1024], F32, tag="mm", name="ps_qk")
                for k in range(NK):
                    nc.tensor.matmul(
                        ps[:, 0:512],
                        wt[k][:, P * p:P * (p + 1)],
                        xt[k][:, 512 * c:512 * (c + 1)],
                        start=(k == 0),
                        stop=(k == NK - 1),
                    )
                return ps

            def emit_qkv_evict(c, i, ps):
                # bias-add eviction on DVE (keeps ACT free for exp); when
                # deferred a block behind the matmuls it never stalls
                with nc.allow_low_precision(reason="qkT stored bf16"):
                    nc.vector.tensor_scalar_add(
                        out=qkT[i][:, 512 * c:512 * (c + 1)],
                        in0=ps[:, 0:512],
                        scalar1=bqk_sb[:, i:i + 1],
                    )

            def emit_qkv_group(c, i):
                ps = emit_qkv_group_mm(c, i)
                emit_qkv_evict(c, i, ps)

            def emit_v_block(tb):
                # v natural [t, d] + bias, packed into pair blocks; the
                # eviction arithmetic runs on gpsimd so the DVE queue stays
                # free for attention's mask multiplies
                ps = psMM.tile([P, 1024], F32, tag="mm", name="ps_v")
                for k in range(NK):
                    nc.tensor.matmul(
                        ps[:, 0:GC],
                        xt[k][:, P * tb:P * (tb + 1)],
                        wvt[k][:],
                        start=(k == 0),
                        stop=(k == NK - 1),
                    )
                v3 = vsb[tb].rearrange("p (b e) -> p b e", e=VPB)
                ps4 = ps[:, 0:GC].rearrange("p (b o d) -> p b o d", o=2, d=HS)
                with nc.allow_low_precision(reason="v stored bf16"):
                    nc.vector.tensor_copy(out=v3[:, :, 0:HS],
                                          in_=ps4[:, :, 0, :])
                    nc.vector.tensor_copy(out=v3[:, :, 2 * HS:3 * HS],
                                          in_=ps4[:, :, 1, :])
                nc.gpsimd.memset(v3[:, :, HS:HS + 1], 1.0)
                nc.gpsimd.memset(v3[:, :, HS + 1:2 * HS], 0.0)

            def emit_norm_pair(hp, ypsA, ypsB):
                # Even head: sums at ypsA row 64 -> stage to SBUF (DVE,
                # partition-aligned), DMA to partition 0 (gpsimd custom ops
                # need base-0 operands on HW), reciprocal at base 0, gpsimd
                # partition_broadcast to rows 0-63.  Odd head: sums at ypsB
                # row 0 -> reciprocal at base 0, gpsimd-broadcast across all
                # 128 rows (base-0 dst), multiply lane-aligned at base 64.
                st = work.tile([HS + 1, 512], F32, tag="st", name="st")
                nc.vector.tensor_copy(out=st[HS:HS + 1, :],
                                      in_=ypsA[HS:HS + 1, :])
                stE = work.tile([1, 512], F32, tag="stE", name="stE")
                nc.sync.dma_start(stE[:], st[HS:HS + 1, :])
                rtE = work.tile([1, 512], F32, tag="rtE", name="rtE")
                nc.vector.reciprocal_approx_fast(out=rtE[:], in_=stE[:])
                rbiA = work.tile([HS, 512], F32, tag="rbiA", name="rbiA")
                nc.gpsimd.partition_broadcast(rbiA[:], rtE[:], channels=HS)

                stB = work.tile([1, 512], F32, tag="stB", name="stB")
                nc.vector.tensor_copy(out=stB[:], in_=ypsB[0:1, :])
                rtB = work.tile([1, 512], F32, tag="rtB", name="rtB")
                nc.vector.reciprocal_approx_fast(out=rtB[:], in_=stB[:])
                rbiB = work.tile([P, 512], F32, tag="rbiB", name="rbiB")
                nc.gpsimd.partition_broadcast(rbiB[0:P, :], rtB[:],
                                              channels=P)

                yt = ytp.tile([P, 512], MM, tag=f"ytp{hp}", name=f"ytp{hp}")
                nc.vector.tensor_mul(out=yt[0:HS, :], in0=ypsA[0:HS, :],
                                     in1=rbiA[:])
                nc.vector.tensor_mul(out=yt[HS:P, :], in0=ypsB[HS:P, :],
                                     in1=rbiB[HS:P, :])
                return yt

            def emit_norm_pair_tail(hp, ypsA, ypsB):
                # PE-broadcast variant for the very end of the kernel: the
                # chain is ~2.5us shorter than the gpsimd one and there is
                # no S-stream whose PSUM rotation it could disturb
                st = work.tile([HS + 1, 512], MM, tag="stT", name="stT")
                with nc.allow_low_precision(reason="denom staged bf16"):
                    nc.vector.tensor_copy(out=st[HS:HS + 1, :],
                                          in_=ypsA[HS:HS + 1, :])
                rb = psMM.tile([P, 1024], F32, tag="mm", name="rb")
                nc.tensor.matmul(rb[0:HS, 0:512], ones65[HS:HS + 1, :],
                                 st[HS:HS + 1, :], start=True, stop=True)
                rbiA = work.tile([HS, 512], F32, tag="rbiA", name="rbiA")
                nc.vector.reciprocal_approx_fast(out=rbiA[:],
                                                 in_=rb[0:HS, 0:512])

                stB = work.tile([1, 512], F32, tag="stB", name="stB")
                nc.vector.tensor_copy(out=stB[:], in_=ypsB[0:1, :])
                rtB = work.tile([1, 512], F32, tag="rtB", name="rtB")
                nc.vector.reciprocal_approx_fast(out=rtB[:], in_=stB[:])
                rtBb = work.tile([1, 512], MM, tag="rtBb", name="rtBb")
                with nc.allow_low_precision(reason="denom staged bf16"):
                    nc.vector.tensor_copy(out=rtBb[:], in_=rtB[:])
                nc.tensor.matmul(rb[HS:P, 512:1024], ones65[0:1, :],
                                 rtBb[:], start=True, stop=True,
                                 tile_position=(0, HS))
                rbiB = work.tile([P, 512], F32, tag="rbiB", name="rbiB")
                nc.vector.tensor_copy(out=rbiB[HS:P, :],
                                      in_=rb[HS:P, 512:1024])

                yt = ytp.tile([P, 512], MM, tag="ytpT", name="ytpT")
                nc.vector.tensor_mul(out=yt[0:HS, :], in0=ypsA[0:HS, :],
                                     in1=rbiA[:])
                nc.vector.tensor_mul(out=yt[HS:P, :], in0=ypsB[HS:P, :],
                                     in1=rbiB[HS:P, :])
                return yt

            def emit_proj_mm(c, ytiles, tq):
                # one token-block of the output projection (K=128 over the
                # 3 pair tiles); exactly one psMM allocation
                pps = psMM.tile([P, 1024], F32, tag="mm", name="ps_o")
                for hp in range(3):
                    for n0, nn in ((0, 512), (512, 256)):
                        nc.tensor.matmul(
                            pps[:, n0:n0 + nn],
                            ytiles[hp][:, P * tq:P * (tq + 1)],
                            wph[hp][:, n0:n0 + nn],
                            start=(hp == 0),
                            stop=(hp == 2),
                        )
                return pps

            def emit_proj_evict(c, tq, pps, on_act=False):
                tb = 4 * c + tq
                ot = work.tile([P, C], F32, tag="ot", name="ot")
                if on_act:
                    nc.scalar.copy(out=ot[:], in_=pps[:, 0:C])
                else:
                    nc.vector.tensor_copy(out=ot[:], in_=pps[:, 0:C])
                nc.sync.dma_start(outv[tb], ot[:])

            def emit_proj_tb(c, ytiles, tq, on_act=False):
                pps = emit_proj_mm(c, ytiles, tq)
                emit_proj_evict(c, tq, pps, on_act)

            # ---- flat block stream ----
            # All (chunk, pair, key-block) S/exp/mask blocks form one
            # continuous stream; PV runs two blocks behind S (its es gate
            # is stale by then), and the stream crosses pair and chunk
            # boundaries without draining, so the ACT exp pipeline never
            # starves at a boundary.  QKV groups for later pairs/chunks,
            # v-blocks and projections are injected between blocks (each
            # is one PSUM allocation whose ~1.3us of matmuls bridge the
            # exp gate its slot flip creates); their evictions run a block
            # deferred so they never head-of-line block the evicting
            # engine.
            ytiles_by_chunk = [[None] * 3 for _ in range(NQCH)]
            pair_tiles = {}
            pending = []
            pend_ev = []

            def emit_pv_from(desc):
                c, hp, j, qs, es, vpj, jlast = desc
                key = (c, hp)
                if key not in pair_tiles:
                    ypsA = psY.tile([HS + 2, 512], F32, tag="y", name="ypsA")
                    ypsB = psY.tile([P, 512], F32, tag="y", name="ypsB")
                    pair_tiles[key] = (ypsA, ypsB)
                ypsA, ypsB = pair_tiles[key]
                nc.tensor.matmul(
                    ypsA[:, qs:512], vpj[:, 0:HS + 2], es[:, qs:512],
                    start=(j == 0), stop=(j == jlast),
                )
                nc.tensor.matmul(
                    ypsB[:, qs:512], vpj[:, HS:VPB], es[:, 512 + qs:1024],
                    start=(j == 0), stop=(j == jlast),
                )
                if j == jlast and key != (NQCH - 1, 2):
                    ytiles_by_chunk[c][hp] = emit_norm_pair(hp, ypsA, ypsB)

            def emit_block(c, hp, j, jlast):
                qTA = qkT[hp][0:HS, :]
                qTB = qkT[hp][HS:P, :]
                kTA = qkT[3 + hp][0:HS, :]
                kTB = qkT[3 + hp][HS:P, :]
                m = j - 4 * c
                qs = P * m if m > 0 else 0
                sps = psMM.tile([P, 1024], F32, tag="mm", name="ps_s")
                es = work.tile([P, 1024], MM, tag="es", name="es", bufs=5)
                nc.tensor.matmul(
                    sps[:, qs:512],
                    kTA[:, P * j:P * (j + 1)],
                    qTA[:, 512 * c + qs:512 * (c + 1)],
                    start=True, stop=True,
                )
                nc.tensor.matmul(
                    sps[:, 512 + qs:1024],
                    kTB[:, P * j:P * (j + 1)],
                    qTB[:, 512 * c + qs:512 * (c + 1)],
                    start=True, stop=True,
                )
                if qs > 0:
                    es2 = es.rearrange("p (u n) -> p u n", n=512)
                    sp2 = sps.rearrange("p (u n) -> p u n", n=512)
                    nc.scalar.activation(
                        out=es2[:, :, qs:512], in_=sp2[:, :, qs:512],
                        func=mybir.ActivationFunctionType.Exp,
                        scale=1.0 / 8.0)
                else:
                    nc.scalar.activation(
                        out=es[:], in_=sps[:],
                        func=mybir.ActivationFunctionType.Exp,
                        scale=1.0 / 8.0)
                if m >= 0:
                    es2 = es.rearrange("p (u n) -> p u n", n=512)
                    mk2 = mask_sb.rearrange("p (u n) -> p u n", n=P)
                    nc.vector.tensor_mul(
                        out=es2[:, :, qs:qs + P],
                        in0=es2[:, :, qs:qs + P], in1=mk2[:])
                vpj = vsb[j].rearrange("p (b e) -> p b e", e=VPB)[:, hp, :]
                while pend_ev:
                    pend_ev.pop(0)()
                if len(pending) >= 2:
                    emit_pv_from(pending.pop(0))
                pending.append((c, hp, j, qs, es, vpj, jlast))

            # ---- main schedule ----
            # serial head: just enough QKV for chunk-0 pair-0
            emit_qkv_group(0, 0)
            emit_qkv_group(0, 3)
            for tq in range(4):
                emit_v_block(tq)

            def G(c, i):
                def f():
                    ps = emit_qkv_group_mm(c, i)
                    return lambda: emit_qkv_evict(c, i, ps)
                return f

            def V(tb):
                def f():
                    emit_v_block(tb)
                    return None
                return f

            def PP(c, tq0):
                # proj token-blocks in pairs (two psMM allocations keep the
                # S rotation parity)
                def f():
                    yy = ytiles_by_chunk[c]
                    p1 = emit_proj_mm(c, yy, tq0)
                    p2 = emit_proj_mm(c, yy, tq0 + 1)

                    def ev():
                        emit_proj_evict(c, tq0, p1)
                        emit_proj_evict(c, tq0 + 1, p2)
                    return ev
                return f

            for c in range(NQCH):
                inject = [G(c, 1), G(c, 4), G(c, 2), G(c, 5)]
                if c + 1 < NQCH:
                    inject += [G(c + 1, 0), G(c + 1, 3)]
                    inject += [V(4 * (c + 1) + tq) for tq in range(4)]
                if c >= 1:
                    inject += [PP(c - 1, 0), PP(c - 1, 2)]
                jlast = 4 * c + 3
                nblocks = 3 * (jlast + 1)
                stride = max(1, nblocks // len(inject))
                bcount = 0
                for hp in range(3):
                    for j in range(jlast + 1):
                        emit_block(c, hp, j, jlast)
                        bcount += 1
                        if inject and bcount % stride == 0:
                            ev = inject.pop(0)()
                            if ev is not None:
                                pend_ev.append(ev)
                while inject:
                    ev = inject.pop(0)()
                    if ev is not None:
                        pend_ev.append(ev)

            # stream drain: last two PVs, tail norm, last projection
            while pend_ev:
                pend_ev.pop(0)()
            while pending:
                emit_pv_from(pending.pop(0))
            ypsA, ypsB = pair_tiles[(NQCH - 1, 2)]
            ytiles_by_chunk[NQCH - 1][2] = emit_norm_pair_tail(2, ypsA, ypsB)
            for tq in range(4):
                emit_proj_tb(NQCH - 1, ytiles_by_chunk[NQCH - 1], tq,
                             on_act=(tq % 2 == 0))

    nc.compile()
    return nc


_nc_cache = None
last_results = None


def _get_nc():
    global _nc_cache
    if _nc_cache is None:
        _nc_cache = _build_nc()
    return _nc_cache


def make_in_maps(x, W_attn, b_attn, W_proj):
    x = np.asarray(x, np.float32)
    W_attn = np.asarray(W_attn, np.float32)
    b_attn = np.asarray(b_attn, np.float32)
    W_proj = np.asarray(W_proj, np.float32)

    kk, qq = np.meshgrid(np.arange(P), np.arange(P), indexing="ij")
    mask = np.tile((qq >= kk).astype(NP_MM), (1, 2))

    in_maps = []
    for core in range(NCORES):
        b, g = divmod(core, 2)
        hs = slice(GC * g, GC * (g + 1))
        bq = b_attn[0:C][hs]
        bk = b_attn[C:2 * C][hs]
        bqk = np.stack(
            [bq[P * p:P * (p + 1)] for p in range(3)]
            + [bk[P * p:P * (p + 1)] for p in range(3)],
            axis=1,
        ).astype(np.float32)
        in_maps.append({
            "xT": np.ascontiguousarray(x[b].T).astype(NP_MM),
            "wq": np.ascontiguousarray(W_attn[:, 0:C][:, hs]).astype(NP_MM),
            "wk": np.ascontiguousarray(W_attn[:, C:2 * C][:, hs]).astype(NP_MM),
            "wv": np.ascontiguousarray(W_attn[:, 2 * C:3 * C][:, hs]).astype(NP_MM),
            "wp": np.ascontiguousarray(W_proj[hs, :]).astype(NP_MM),
            "bqk": np.ascontiguousarray(bqk),
            "mask": mask,
        })
    return in_maps


def kernel(x, W_attn, b_attn, W_proj, b_proj, _trace=False):
    global last_results
    nc = _get_nc()
    in_maps = make_in_maps(x, W_attn, b_attn, W_proj)
    res = run_bass_kernel_spmd(nc, in_maps, list(range(NCORES)), trace=_trace)
    last_results = res
    out = np.zeros((B, T, C), np.float32)
    for core in range(NCORES):
        out[core // 2] += res.results[core]["out"]
    # v-bias contribution (sum_k es*(v+bv) normalizes to y + bv) plus b_proj
    bias = np.asarray(b_proj, np.float32) + (
        np.asarray(b_attn, np.float32)[2 * C:3 * C]
        @ np.asarray(W_proj, np.float32))
    out += bias[None, None, :]
    return out
